# revision 32
# baseline (speedup 1.0000x reference)
"""Trainium2 Bass kernel for the decomposable-attention "Attend" block.

reference:
    f_A = relu(relu(A@W1+b1)@W2+b2); f_B likewise      (bs, t, hid)
    e = f_A @ f_B^T                                     (bs, ta, tb)
    beta  = softmax(e, -1) @ B                          (bs, ta, emb)
    alpha = softmax(e^T, -1) @ A                        (bs, tb, emb)
    returns (beta, alpha)

Sharding: data-parallel over batch (16 batches / 8 cores = 2 per core);
W1/b1/W2/b2 replicated.

Single-exp softmax scheme (the core trick): e is computed ONCE per
batch, in [a, c] chunks (stationary = FA a-blocks, moving = FB halves),
with a single GLOBAL stabilizer X = G0 - 44 (G0 = max of e-chunk 0; the
gap G - G0 is tiny, and +-44 centers the fp32 exp range: max entry
~e^48, min useful column-max ~e^-43).
    V[a,c] = exp(e[a,c] - G0 + 44)        (bf16, [a, c] layout)
Because the stabilizer is one constant, V serves BOTH softmaxes after
normalization:
  alpha = diag(1/Z'[c]) V^T A   with Z'[c] = column sums of V, obtained
          free from an all-ones column appended to the A moving operand.
  beta  = diag(1/Z*[a]) S B     with S = V^T (64 bf16 PE transposes,
          1 cycle/row) and Z*[a] = row sums of V from the exp
          accumulator.
This removes the whole second e matmul pass, its stats, and half the
exp transposes of the two-pass formulation.

Other structural points:
  - A/B/W1/W2 DRAM params and their SBUF tiles are DECLARED float32r
    (same 4-byte layout as f32; the PE truncates on load), so there are
    no f32->f32r rounding copies at all and the input-prep transposes
    run at the f32r rate (1.5 c/row).  The BIR verifier requires the
    PRODUCER of any f32r-matmul operand to be f32r-typed - bitcasting
    an f32 tile is rejected - hence typed-at-declaration.
  - MLP + e matmuls stay f32r (bf16 MLP measured 1.8e-2 rel err - too
    close to the 2e-2 gate).  Only the softmax-weight matmuls (V/S
    stationary, A/B natural moving) are bf16.
  - Matmuls are stationary-paired: each stationary tile feeds two
    moving halves accumulating in two psum banks.
  - A/B natural-layout bf16 copies (An/Bn) are made during input prep;
    A and B are DMA'd only once per batch.  S_ca reuses the At/Bt slot
    (dead after L1(B); the next batch's At is written only after
    alpha), V reuses the H slot.
  - The e-region PE stream interleaves e-chunks, transpose groups and
    beta groups so exp latency and DVE copies never stall the PE; the
    next batch's A prep + L1(A) run in the current batch's tail, with
    its B pair loads pre-issued there too.
  - Alpha's psum groups alternate between the two psum pools (the
    transpose pool is idle in that phase) to ride out output-store
    latency with a single output staging buffer.
  - One transpose per psum bank: pairing two 128x128 transposes into
    one bank measured ~50us SLOWER on hardware (bank write
    serialization) despite being neutral in the cost model.
  - W1 loads on the ACT hwdge queue at body top (idle there), in
    parallel with the A pair loads on the SP queue; W2 follows on SP
    under L1(A) cover.  Everything else stays on SP - engine-queue
    DMAs occupy that engine's sequencer for ~descriptor-gen time, so
    the ACT queue is only used when ACT has nothing better to do.

DMA instructions carry a large fixed cost on this runtime (~5us), so
transfers are batched: paired 256-token input loads, whole-tensor
weight loads, paired 256-row output stores.
"""
import sys

sys.path.insert(0, "/opt/trn_rl_repo")

import numpy as np

N_CORES = 8
B_SZ, T, EMB, HID = 16, 1024, 768, 1024
BL = B_SZ // N_CORES  # batches per core
P = 128
EC = EMB // P   # 6 emb chunks
HC = HID // P   # 8 hid chunks
TC = T // P     # 8 token chunks
OFFS = 44.0     # exp-range centering offset

_CACHE = {}


def _split_multi_waits(nc):
    """This walrus build accepts only ONE sync-wait per instruction; Tile
    attaches one wait per producer semaphore. Split any multi-wait
    instruction into single-wait NoOps (same engine, just before it) plus
    the original carrying the last wait."""
    from concourse import mybir

    n = 0
    for fn in nc.m.functions:
        for bb in fn.blocks:
            il = list(bb.instructions)
            out = []
            changed = False
            for ins in il:
                si = getattr(ins, "sync_info", None)
                waits = list(si.on_wait) if (si is not None and si.on_wait) else []
                if len(waits) > 1 and ins.engine != mybir.EngineType.Unassigned:
                    for w in waits[:-1]:
                        n += 1
                        nop = mybir.InstNoOp(name=f"nopw-{n}", ins=[], outs=[])
                        nop.engine = ins.engine
                        nop.sync_info = mybir.SyncInfo(on_wait=[w], on_update=[])
                        out.append(nop)
                    si.on_wait = waits[-1:]
                    changed = True
                out.append(ins)
            if changed:
                bb.instructions = out
    return n


def _build_nc(reps=1, loop_reps=1):
    import concourse.bass as bass
    import concourse.tile as tile
    from concourse import bass_isa, mybir
    from concourse.masks import make_identity
    from contextlib import ExitStack, nullcontext

    f32 = mybir.dt.float32
    f32r = mybir.dt.float32r
    bf16 = mybir.dt.bfloat16
    AF = mybir.ActivationFunctionType
    AX = mybir.AxisListType
    OP = mybir.AluOpType

    nc = bass.Bass(dynamic_dma_scratch_size=512)
    A_d = nc.declare_dram_parameter("A", [BL, T, EMB], f32r, isOutput=False)
    B_d = nc.declare_dram_parameter("B", [BL, T, EMB], f32r, isOutput=False)
    W1_d = nc.declare_dram_parameter("W1", [EMB, HID], f32r, isOutput=False)
    b1_d = nc.declare_dram_parameter("b1", [HID], f32, isOutput=False)
    W2_d = nc.declare_dram_parameter("W2", [HID, HID], f32r, isOutput=False)
    b2_d = nc.declare_dram_parameter("b2", [HID], f32, isOutput=False)
    beta_d = nc.declare_dram_parameter("beta", [BL, T, EMB], f32, isOutput=True)
    alpha_d = nc.declare_dram_parameter("alpha", [BL, T, EMB], f32, isOutput=True)

    with tile.TileContext(nc) as tc, ExitStack() as ctx:
        main = ctx.enter_context(tc.tile_pool(name="main", bufs=1))
        nat = ctx.enter_context(tc.tile_pool(name="nat", bufs=2))
        obp = ctx.enter_context(tc.tile_pool(name="obp", bufs=1))
        stats = ctx.enter_context(tc.tile_pool(name="stats", bufs=8))
        psA = ctx.enter_context(tc.tile_pool(name="psA", bufs=4, space="PSUM"))
        psT = ctx.enter_context(tc.tile_pool(name="psT", bufs=4, space="PSUM"))

        idf = main.tile([P, P], f32, tag="id0")
        make_identity(nc, idf[:])
        idr = main.tile([P, P], f32r, tag="idf")
        nc.vector.tensor_copy(idr[:], idf[:])
        idb = main.tile([P, P], bf16, tag="idb")
        nc.vector.tensor_copy(idb[:], idf[:])
        ones1 = main.tile([1, P], f32, tag="ones1")
        nc.gpsimd.memset(ones1[:], 1.0)
        b1t = main.tile([P, HC], f32, tag="b1t")
        nc.sync.dma_start(b1t[:], b1_d[:].rearrange("(o p) -> p o", p=P))
        b2t = main.tile([P, HC], f32, tag="b2t")
        nc.sync.dma_start(b2t[:], b2_d[:].rearrange("(o p) -> p o", p=P))

        # one DMA brings TWO 128-token chunks into a nat buffer; PE
        # transposes them into feature-major f32r Xt; the natural layout
        # is also converted to bf16 (An/Bn) for the out-stage
        def pair_dma(X_d, b, tp):
            an = nat.tile([P, 2, EMB], f32r, tag="nat")
            nc.sync.dma_start(
                an[:], X_d[b, tp * 2 * P:(tp + 1) * 2 * P, :]
                .rearrange("(c p) e -> p c e", p=P))
            return an

        def pair_consume(an, Xt_t, tp, NatT):
            nc.vector.tensor_copy(NatT[:, tp * 2:(tp + 1) * 2, 0:EMB], an[:])
            for c in range(2):
                t = tp * 2 + c
                for ec in range(EC):
                    pt = psT.tile([P, P], f32r, tag="tp")
                    nc.tensor.transpose(
                        pt[:], an[:, c, ec * P:(ec + 1) * P], idr[:])
                    nc.vector.tensor_copy(Xt_t[:, ec, t * P:(t + 1) * P], pt[:])

        def prep_pair(X_d, b, Xt_t, tp, NatT):
            pair_consume(pair_dma(X_d, b, tp), Xt_t, tp, NatT)

        # stationary-paired layer: both 512-halves per weight tile
        def layer(Wf, bt, Xin, Hout, kc, weave=None):
            for m in range(HC):
                ps0 = psA.tile([P, 512], f32, tag="acc")
                ps1 = psA.tile([P, 512], f32, tag="acc")
                for ko in range(kc):
                    st = Wf[:, ko, m * P:(m + 1) * P]
                    nc.tensor.matmul(ps0[:], st, Xin[:, ko, 0:512],
                                     start=(ko == 0), stop=(ko == kc - 1))
                    nc.tensor.matmul(ps1[:], st, Xin[:, ko, 512:1024],
                                     start=(ko == 0), stop=(ko == kc - 1))
                nc.scalar.activation(Hout[:, m, 0:512], ps0[:],
                                     AF.Relu, bias=bt[:, m:m + 1])
                nc.scalar.activation(Hout[:, m, 512:1024], ps1[:],
                                     AF.Relu, bias=bt[:, m:m + 1])
                if weave:
                    weave.pop(0)()

        # single-half layer (prologue: lets L1(A) start after 2 pairs)
        def layer_half(Wf, bt, Xin, Hout, kc, tf, weave=None):
            for m in range(HC):
                ps = psA.tile([P, 512], f32, tag="acc")
                for ko in range(kc):
                    nc.tensor.matmul(
                        ps[:], Wf[:, ko, m * P:(m + 1) * P],
                        Xin[:, ko, tf * 512:(tf + 1) * 512],
                        start=(ko == 0), stop=(ko == kc - 1))
                nc.scalar.activation(Hout[:, m, tf * 512:(tf + 1) * 512],
                                     ps[:], AF.Relu, bias=bt[:, m:m + 1])
                if weave:
                    weave.pop(0)()

        loop_ctx = tc.For_i(0, loop_reps, 1) if loop_reps > 1 else nullcontext()
        with loop_ctx:
            for rep in range(reps):
                w1f = main.tile([P, EC, HID], f32r, tag="w1f")
                # ACT hwdge queue: idle at body top, so W1 loads in parallel
                # with the A pair loads on the SP queue; two halves so the
                # first L1(A) chains wait only on the earlier-landing piece
                nc.scalar.dma_start(
                    w1f[:, 0:3, :],
                    W1_d[0:3 * P, :].rearrange("(ko p) h -> p ko h", p=P))
                nc.scalar.dma_start(
                    w1f[:, 3:6, :],
                    W1_d[3 * P:6 * P, :].rearrange("(ko p) h -> p ko h", p=P))
                w2f = main.tile([P, HC, HID], f32r, tag="w2f")

                # prologue: A(b=0) preps; pairs 2/3 + W2 woven around L1(A)
                At = main.tile([P, EC, T], f32r, tag="Xt")
                An = main.tile([P, TC, EMB + 1], bf16, tag="An")
                nc.gpsimd.memset(An[:, :, EMB:EMB + 1], 1.0)
                prep_pair(A_d, 0, At, 0, An)
                prep_pair(A_d, 0, At, 1, An)

                H = main.tile([P, HC, T], f32r, tag="H")
                layer_half(w1f, b1t, At, H, EC, 0)
                prep_pair(A_d, 0, At, 2, An)
                prep_pair(A_d, 0, At, 3, An)
                nc.sync.dma_start(
                    w2f[:], W2_d[:].rearrange("(ko p) h -> p ko h", p=P))
                layer_half(w1f, b1t, At, H, EC, 1)

                nextA = None
                for b in range(BL):
                    if b > 0:
                        At, An, H, bn0, bn1 = nextA
                    # L2(A): weave B preps (Bt reuses the At slot; At is
                    # dead after L1(A)).  DMA first, consume two hooks later
                    # so the transfer is in flight during the matmuls.
                    Bt = main.tile([P, EC, T], f32r, tag="Xt")
                    Bn = main.tile([P, TC, EMB], bf16, tag="Bn")
                    FA = main.tile([P, HC, T], f32r, tag="FA")
                    bst = {}

                    def bdma(tp):
                        bst[tp] = pair_dma(B_d, b, tp)

                    def bcon(tp):
                        pair_consume(bst.pop(tp), Bt, tp, Bn)

                    if b == 0:
                        bdma(0)
                        bdma(1)
                    else:
                        bst[0], bst[1] = bn0, bn1
                    layer(w2f, b2t, H, FA, HC, weave=[
                        lambda: (bcon(0), bdma(2)), lambda: (bcon(1), bdma(3)),
                        lambda: bcon(2), lambda: bcon(3),
                        lambda: None, lambda: None,
                        lambda: None, lambda: None])

                    H2 = main.tile([P, HC, T], f32r, tag="H")
                    FB = main.tile([P, HC, T], f32r, tag="FB")
                    layer(w1f, b1t, Bt, H2, EC)
                    layer(w2f, b2t, H2, FB, HC)

                    # ---- e phase: one matmul pass, global stabilizer ----
                    Vx = main.tile([P, TC, T], bf16, tag="H")
                    Sca = main.tile([P, TC, T], bf16, tag="Xt")
                    rzb = main.tile([P, TC], f32, tag="rzb")
                    biast = main.tile([P, 1], f32, tag="biast")

                    def e_chunk(m, weave=None):
                        ps0 = psA.tile([P, 512], f32, tag="acc")
                        ps1 = psA.tile([P, 512], f32, tag="acc")
                        for k in range(HC):
                            st = FA[:, k, m * P:(m + 1) * P]
                            nc.tensor.matmul(ps0[:], st, FB[:, k, 0:512],
                                             start=(k == 0), stop=(k == HC - 1))
                            nc.tensor.matmul(ps1[:], st, FB[:, k, 512:1024],
                                             start=(k == 0), stop=(k == HC - 1))
                            if weave:
                                weave.pop(0)()
                        if m == 0:
                            # chunk-0 global max -> bias = OFFS - G0 on all
                            # partitions (gpsimd cross-partition reduce +
                            # broadcast; off the PE critical path)
                            r0 = stats.tile([P, 1], f32, tag="st")
                            r1 = stats.tile([P, 1], f32, tag="st")
                            nc.vector.tensor_reduce(r0[:], ps0[:], axis=AX.X,
                                                    op=OP.max)
                            nc.vector.tensor_reduce(r1[:], ps1[:], axis=AX.X,
                                                    op=OP.max)
                            rM = stats.tile([P, 1], f32r, tag="rm")
                            nc.vector.tensor_tensor(rM[:], r0[:], r1[:], OP.max)
                            # cross-partition max: PE transpose -> free-dim
                            # reduce -> (44 - G0) -> ones-row matmul broadcast
                            ptm = psT.tile([1, P], f32r, tag="tp")
                            nc.tensor.transpose(ptm[:], rM[:], idr[:])
                            g0 = stats.tile([1, 1], f32, tag="g0")
                            nc.vector.tensor_reduce(g0[:], ptm[:], axis=AX.X,
                                                    op=OP.max)
                            gb = stats.tile([1, 1], f32, tag="g0")
                            nc.vector.tensor_scalar(gb[:], g0[:], -1.0, OFFS,
                                                    OP.mult, OP.add)
                            gps = psT.tile([P, 1], f32, tag="tp")
                            nc.tensor.matmul(gps[:], ones1[:], gb[:],
                                             start=True, stop=True)
                            nc.vector.tensor_copy(biast[:], gps[:])
                        zz0 = stats.tile([P, 1], f32, tag="st")
                        zz1 = stats.tile([P, 1], f32, tag="st")
                        nc.scalar.activation(Vx[:, m, 0:512], ps0[:], AF.Exp,
                                             bias=biast[:], accum_out=zz0[:])
                        nc.scalar.activation(Vx[:, m, 512:1024], ps1[:], AF.Exp,
                                             bias=biast[:], accum_out=zz1[:])
                        zs = stats.tile([P, 1], f32, tag="st")
                        nc.vector.tensor_tensor(zs[:], zz0[:], zz1[:], OP.add)
                        nc.vector.reciprocal(rzb[:, m:m + 1], zs[:])

                    def t_ops(j):
                        # V[a-block j, c-block i] -> S_ca[c-chunk i, a-cols j]
                        ops = []
                        for i in range(TC):
                            def one(i=i):
                                pt = psT.tile([P, P], bf16, tag="tp", name="pt")
                                nc.tensor.transpose(
                                    pt[:], Vx[:, j, i * P:(i + 1) * P], idb[:])
                                nc.vector.tensor_copy(
                                    Sca[:, i, j * P:(j + 1) * P], pt[:])
                            ops.append(one)
                        return ops

                    ob_state = {}

                    def out_group(stT, mvT, oc, Out_d, w1, rz_ap, weave=None,
                                  alt=False, last=False):
                        pp = psT if alt else psA
                        tg = "tp" if alt else "acc"
                        ps0 = pp.tile([P, 384], f32, tag=tg, name="ps0")
                        ps1 = pp.tile([P, w1], f32, tag=tg, name="ps1")
                        for ck in range(TC):
                            st = stT[:, ck, oc * P:(oc + 1) * P]
                            nc.tensor.matmul(ps0[:], st, mvT[:, ck, 0:384],
                                             start=(ck == 0), stop=(ck == TC - 1))
                            nc.tensor.matmul(ps1[:], st, mvT[:, ck, 384:384 + w1],
                                             start=(ck == 0), stop=(ck == TC - 1))
                            if weave:
                                weave.pop(0)()
                        if rz_ap is None:
                            rz = stats.tile([P, 1], f32, tag="st")
                            nc.vector.reciprocal(rz[:], ps1[:, 384:385])
                        else:
                            rz = rz_ap
                        if oc % 2 == 0:
                            ob_state["ob"] = obp.tile([P, 2, EMB], f32, tag="ob",
                                                      name="ob")
                        ob = ob_state["ob"]
                        j2 = oc % 2
                        nc.scalar.activation(ob[:, j2, 0:384], ps0[:],
                                             AF.Copy, scale=rz[:])
                        nc.scalar.activation(ob[:, j2, 384:768], ps1[:, 0:384],
                                             AF.Copy, scale=rz[:])
                        if oc % 2 == 1:
                            if last:
                                nc.sync.dma_start(
                                    Out_d[b, (oc - 1) * P:oc * P, :]
                                    .rearrange("(c p) e -> p c e", p=P),
                                    ob[:, 0:1, :])
                                nc.scalar.dma_start(
                                    Out_d[b, oc * P:(oc + 1) * P, :]
                                    .rearrange("(c p) e -> p c e", p=P),
                                    ob[:, 1:2, :])
                            else:
                                nc.sync.dma_start(
                                    Out_d[b, (oc - 1) * P:(oc + 1) * P, :]
                                    .rearrange("(c p) e -> p c e", p=P), ob[:])

                    def beta_group(j, weave=None):
                        out_group(Sca, Bn, j, beta_d, 384, rzb[:, j:j + 1],
                                  weave=weave)

                    # e-region PE stream: transposes hide inside the e/beta
                    # chains; beta(j) follows T(j); exp latency never stalls
                    # the PE.
                    pad = [lambda: None] * 8
                    e_chunk(0)
                    e_chunk(1)
                    e_chunk(2, weave=t_ops(0))
                    beta_group(0, weave=t_ops(1))
                    e_chunk(3)
                    beta_group(1, weave=t_ops(2))
                    e_chunk(4)
                    beta_group(2, weave=t_ops(3))
                    e_chunk(5)
                    beta_group(3, weave=t_ops(4))
                    e_chunk(6)
                    beta_group(4, weave=t_ops(5))
                    e_chunk(7)
                    beta_group(5, weave=t_ops(6))
                    beta_group(6, weave=t_ops(7))
                    beta_group(7)

                    # alpha region; next-batch A pair DMAs + W2 reload launch
                    # under it (S_ca readers - the beta matmuls - are done)
                    a_dmas = []
                    if b + 1 < BL:
                        a_dmas.append(pair_dma(A_d, b + 1, 0))
                        a_dmas.append(pair_dma(A_d, b + 1, 1))
                    for oc in range(TC):
                        out_group(Vx, An, oc, alpha_d, 385, None,
                                  alt=(oc % 2 == 1),
                                  last=(b == BL - 1 and oc == 7))

                    # tail: next-batch A prep + L1(A); An/At/H slots are free
                    # once alpha is done
                    if b + 1 < BL:
                        At_n = main.tile([P, EC, T], f32r, tag="Xt")
                        An_n = main.tile([P, TC, EMB + 1], bf16, tag="An")
                        nc.gpsimd.memset(An_n[:, :, EMB:EMB + 1], 1.0)
                        pair_consume(a_dmas[0], At_n, 0, An_n)
                        d2 = pair_dma(A_d, b + 1, 2)
                        pair_consume(a_dmas[1], At_n, 1, An_n)
                        d3 = pair_dma(A_d, b + 1, 3)
                        pair_consume(d2, At_n, 2, An_n)
                        bdma_n0 = pair_dma(B_d, b + 1, 0)
                        pair_consume(d3, At_n, 3, An_n)
                        bdma_n1 = pair_dma(B_d, b + 1, 1)
                        H_n = main.tile([P, HC, T], f32r, tag="H")
                        layer(w1f, b1t, At_n, H_n, EC)
                        nextA = (At_n, An_n, H_n, bdma_n0, bdma_n1)

    _split_multi_waits(nc)
    return nc


def _get_nc():
    if "nc" not in _CACHE:
        _CACHE["nc"] = _build_nc()
    return _CACHE["nc"]


def kernel(A, B, W1, b1, W2, b2):
    from concourse.bass_utils import run_bass_kernel_spmd

    A = np.asarray(A, dtype=np.float32)
    B = np.asarray(B, dtype=np.float32)
    W1 = np.ascontiguousarray(np.asarray(W1, dtype=np.float32))
    b1 = np.ascontiguousarray(np.asarray(b1, dtype=np.float32))
    W2 = np.ascontiguousarray(np.asarray(W2, dtype=np.float32))
    b2 = np.ascontiguousarray(np.asarray(b2, dtype=np.float32))

    nc = _get_nc()
    in_maps = []
    for c in range(N_CORES):
        in_maps.append({
            "A": np.ascontiguousarray(A[c * BL:(c + 1) * BL]),
            "B": np.ascontiguousarray(B[c * BL:(c + 1) * BL]),
            "W1": W1, "b1": b1, "W2": W2, "b2": b2,
        })
    res = run_bass_kernel_spmd(nc, in_maps, core_ids=list(range(N_CORES)))
    beta = np.concatenate([res.results[c]["beta"] for c in range(N_CORES)], axis=0)
    alpha = np.concatenate([res.results[c]["alpha"] for c in range(N_CORES)], axis=0)
    return beta, alpha


# revision 34
# speedup vs baseline: 3.1970x; 3.1970x over previous
"""Trainium2 Bass kernel for the decomposable-attention "Attend" block.

reference:
    f_A = relu(relu(A@W1+b1)@W2+b2); f_B likewise      (bs, t, hid)
    e = f_A @ f_B^T                                     (bs, ta, tb)
    beta  = softmax(e, -1) @ B                          (bs, ta, emb)
    alpha = softmax(e^T, -1) @ A                        (bs, tb, emb)
    returns (beta, alpha)

Sharding: data-parallel over batch (16 batches / 8 cores = 2 per core);
W1/b1/W2/b2 replicated.

Single-exp softmax scheme (the core trick): e is computed ONCE per
batch, in [a, c] chunks (stationary = FA a-blocks, moving = FB halves),
with a single GLOBAL stabilizer X = G0 - 44 (G0 = max of e-chunk 0; the
gap G - G0 is tiny, and +-44 centers the fp32 exp range: max entry
~e^48, min useful column-max ~e^-43).
    V[a,c] = exp(e[a,c] - G0 + 44)        (bf16, [a, c] layout)
Because the stabilizer is one constant, V serves BOTH softmaxes after
normalization:
  alpha = diag(1/Z'[c]) V^T A   with Z'[c] = column sums of V, obtained
          free from an all-ones column appended to the A moving operand.
  beta  = diag(1/Z*[a]) S B     with S = V^T (64 bf16 PE transposes,
          1 cycle/row) and Z*[a] = row sums of V from the exp
          accumulator.
This removes the whole second e matmul pass, its stats, and half the
exp transposes of the two-pass formulation.

Other structural points:
  - A/B/W1/W2 DRAM params and their SBUF tiles are DECLARED float32r
    (same 4-byte layout as f32; the PE truncates on load), so there are
    no f32->f32r rounding copies at all and the input-prep transposes
    run at the f32r rate (1.5 c/row).  The BIR verifier requires the
    PRODUCER of any f32r-matmul operand to be f32r-typed - bitcasting
    an f32 tile is rejected - hence typed-at-declaration.
  - MLP + e matmuls stay f32r (bf16 MLP measured 1.8e-2 rel err - too
    close to the 2e-2 gate).  Only the softmax-weight matmuls (V/S
    stationary, A/B natural moving) are bf16.
  - Matmuls are stationary-paired: each stationary tile feeds two
    moving halves accumulating in two psum banks.
  - A/B natural-layout bf16 copies (An/Bn) are made during input prep;
    A and B are DMA'd only once per batch.  S_ca reuses the At/Bt slot
    (dead after L1(B); the next batch's At is written only after
    alpha), V reuses the H slot.
  - The e-region PE stream interleaves e-chunks, transpose groups and
    beta groups so exp latency and DVE copies never stall the PE; the
    next batch's A prep + L1(A) run in the current batch's tail, with
    its B pair loads pre-issued there too.
  - Alpha's psum groups alternate between the two psum pools (the
    transpose pool is idle in that phase) to ride out output-store
    latency with a single output staging buffer.
  - One transpose per psum bank: pairing two 128x128 transposes into
    one bank measured ~50us SLOWER on hardware (bank write
    serialization) despite being neutral in the cost model.
  - W1 loads on the ACT hwdge queue at body top (idle there), in
    parallel with the A pair loads on the SP queue; W2 follows on SP
    under L1(A) cover.  Everything else stays on SP - engine-queue
    DMAs occupy that engine's sequencer for ~descriptor-gen time, so
    the ACT queue is only used when ACT has nothing better to do.

DMA instructions carry a large fixed cost on this runtime (~5us), so
transfers are batched: paired 256-token input loads, whole-tensor
weight loads, paired 256-row output stores.
"""
import sys

sys.path.insert(0, "/opt/trn_rl_repo")

import numpy as np

N_CORES = 8
B_SZ, T, EMB, HID = 16, 1024, 768, 1024
BL = B_SZ // N_CORES  # batches per core
P = 128
EC = EMB // P   # 6 emb chunks
HC = HID // P   # 8 hid chunks
TC = T // P     # 8 token chunks
OFFS = 44.0     # exp-range centering offset

_CACHE = {}


def _split_multi_waits(nc):
    """This walrus build accepts only ONE sync-wait per instruction; Tile
    attaches one wait per producer semaphore. Split any multi-wait
    instruction into single-wait NoOps (same engine, just before it) plus
    the original carrying the last wait."""
    from concourse import mybir

    n = 0
    for fn in nc.m.functions:
        for bb in fn.blocks:
            il = list(bb.instructions)
            out = []
            changed = False
            for ins in il:
                si = getattr(ins, "sync_info", None)
                waits = list(si.on_wait) if (si is not None and si.on_wait) else []
                if len(waits) > 1 and ins.engine != mybir.EngineType.Unassigned:
                    for w in waits[:-1]:
                        n += 1
                        nop = mybir.InstNoOp(name=f"nopw-{n}", ins=[], outs=[])
                        nop.engine = ins.engine
                        nop.sync_info = mybir.SyncInfo(on_wait=[w], on_update=[])
                        out.append(nop)
                    si.on_wait = waits[-1:]
                    changed = True
                out.append(ins)
            if changed:
                bb.instructions = out
    return n


def _build_nc(reps=1, loop_reps=1):
    import concourse.bass as bass
    import concourse.tile as tile
    from concourse import bass_isa, mybir
    from concourse.masks import make_identity
    from contextlib import ExitStack, nullcontext

    f32 = mybir.dt.float32
    f32r = mybir.dt.float32r
    bf16 = mybir.dt.bfloat16
    AF = mybir.ActivationFunctionType
    AX = mybir.AxisListType
    OP = mybir.AluOpType

    nc = bass.Bass(dynamic_dma_scratch_size=512)
    A_d = nc.declare_dram_parameter("A", [BL, T, EMB], f32r, isOutput=False)
    B_d = nc.declare_dram_parameter("B", [BL, T, EMB], f32r, isOutput=False)
    W1_d = nc.declare_dram_parameter("W1", [EMB, HID], f32r, isOutput=False)
    b1_d = nc.declare_dram_parameter("b1", [HID], f32, isOutput=False)
    W2_d = nc.declare_dram_parameter("W2", [HID, HID], f32r, isOutput=False)
    b2_d = nc.declare_dram_parameter("b2", [HID], f32, isOutput=False)
    beta_d = nc.declare_dram_parameter("beta", [BL, T, EMB], f32, isOutput=True)
    alpha_d = nc.declare_dram_parameter("alpha", [BL, T, EMB], f32, isOutput=True)

    with tile.TileContext(nc) as tc, ExitStack() as ctx:
        main = ctx.enter_context(tc.tile_pool(name="main", bufs=1))
        nat = ctx.enter_context(tc.tile_pool(name="nat", bufs=2))
        obp = ctx.enter_context(tc.tile_pool(name="obp", bufs=1))
        stats = ctx.enter_context(tc.tile_pool(name="stats", bufs=8))
        psA = ctx.enter_context(tc.tile_pool(name="psA", bufs=4, space="PSUM"))
        psT = ctx.enter_context(tc.tile_pool(name="psT", bufs=4, space="PSUM"))

        idf = main.tile([P, P], f32, tag="id0")
        make_identity(nc, idf[:])
        idr = main.tile([P, P], f32r, tag="idf")
        nc.vector.tensor_copy(idr[:], idf[:])
        idb = main.tile([P, P], bf16, tag="idb")
        nc.vector.tensor_copy(idb[:], idf[:])
        ones1 = main.tile([1, P], f32, tag="ones1")
        nc.gpsimd.memset(ones1[:], 1.0)
        b1t = main.tile([P, HC], f32, tag="b1t")
        nc.sync.dma_start(b1t[:], b1_d[:].rearrange("(o p) -> p o", p=P))
        b2t = main.tile([P, HC], f32, tag="b2t")
        nc.sync.dma_start(b2t[:], b2_d[:].rearrange("(o p) -> p o", p=P))

        # one DMA brings TWO 128-token chunks into a nat buffer; PE
        # transposes them into feature-major f32r Xt; the natural layout
        # is also converted to bf16 (An/Bn) for the out-stage
        def pair_dma(X_d, b, tp, q=None):
            an = nat.tile([P, 2, EMB], f32r, tag="nat")
            (q or nc.sync).dma_start(
                an[:], X_d[b, tp * 2 * P:(tp + 1) * 2 * P, :]
                .rearrange("(c p) e -> p c e", p=P))
            return an

        def pair_consume(an, Xt_t, tp, NatT):
            nc.vector.tensor_copy(NatT[:, tp * 2:(tp + 1) * 2, 0:EMB], an[:])
            for c in range(2):
                t = tp * 2 + c
                for ec in range(EC):
                    pt = psT.tile([P, P], f32r, tag="tp")
                    nc.tensor.transpose(
                        pt[:], an[:, c, ec * P:(ec + 1) * P], idr[:])
                    nc.vector.tensor_copy(Xt_t[:, ec, t * P:(t + 1) * P], pt[:])

        def prep_pair(X_d, b, Xt_t, tp, NatT, q=None):
            pair_consume(pair_dma(X_d, b, tp, q=q), Xt_t, tp, NatT)

        # stationary-paired layer: both 512-halves per weight tile
        def layer(Wf, bt, Xin, Hout, kc, weave=None):
            for m in range(HC):
                ps0 = psA.tile([P, 512], f32, tag="acc")
                ps1 = psA.tile([P, 512], f32, tag="acc")
                for ko in range(kc):
                    st = Wf[:, ko, m * P:(m + 1) * P]
                    nc.tensor.matmul(ps0[:], st, Xin[:, ko, 0:512],
                                     start=(ko == 0), stop=(ko == kc - 1))
                    nc.tensor.matmul(ps1[:], st, Xin[:, ko, 512:1024],
                                     start=(ko == 0), stop=(ko == kc - 1))
                nc.scalar.activation(Hout[:, m, 0:512], ps0[:],
                                     AF.Relu, bias=bt[:, m:m + 1])
                nc.scalar.activation(Hout[:, m, 512:1024], ps1[:],
                                     AF.Relu, bias=bt[:, m:m + 1])
                if weave:
                    weave.pop(0)()

        # single-half layer (prologue: lets L1(A) start after 2 pairs)
        def layer_half(Wf, bt, Xin, Hout, kc, tf, weave=None):
            for m in range(HC):
                ps = psA.tile([P, 512], f32, tag="acc")
                for ko in range(kc):
                    nc.tensor.matmul(
                        ps[:], Wf[:, ko, m * P:(m + 1) * P],
                        Xin[:, ko, tf * 512:(tf + 1) * 512],
                        start=(ko == 0), stop=(ko == kc - 1))
                nc.scalar.activation(Hout[:, m, tf * 512:(tf + 1) * 512],
                                     ps[:], AF.Relu, bias=bt[:, m:m + 1])
                if weave:
                    weave.pop(0)()

        loop_ctx = tc.For_i(0, loop_reps, 1) if loop_reps > 1 else nullcontext()
        with loop_ctx:
            for rep in range(reps):
                w1f = main.tile([P, EC, HID], f32r, tag="w1f")
                # ACT hwdge queue: idle at body top, so W1 loads in parallel
                # with the A pair loads on the SP queue; two halves so the
                # first L1(A) chains wait only on the earlier-landing piece
                nc.scalar.dma_start(
                    w1f[:, 0:3, :],
                    W1_d[0:3 * P, :].rearrange("(ko p) h -> p ko h", p=P))
                nc.scalar.dma_start(
                    w1f[:, 3:6, :],
                    W1_d[3 * P:6 * P, :].rearrange("(ko p) h -> p ko h", p=P))
                w2f = main.tile([P, HC, HID], f32r, tag="w2f")

                # prologue: A(b=0) preps; pairs 2/3 + W2 woven around L1(A)
                At = main.tile([P, EC, T], f32r, tag="Xt")
                An = main.tile([P, TC, EMB + 1], bf16, tag="An")
                nc.gpsimd.memset(An[:, :, EMB:EMB + 1], 1.0)
                prep_pair(A_d, 0, At, 0, An)
                prep_pair(A_d, 0, At, 1, An)

                H = main.tile([P, HC, T], f32r, tag="H")
                layer_half(w1f, b1t, At, H, EC, 0)
                prep_pair(A_d, 0, At, 2, An)
                prep_pair(A_d, 0, At, 3, An)
                nc.sync.dma_start(
                    w2f[:], W2_d[:].rearrange("(ko p) h -> p ko h", p=P))
                layer_half(w1f, b1t, At, H, EC, 1)

                nextA = None
                for b in range(BL):
                    if b > 0:
                        At, An, H, bn0, bn1 = nextA
                    # L2(A): weave B preps (Bt reuses the At slot; At is
                    # dead after L1(A)).  DMA first, consume two hooks later
                    # so the transfer is in flight during the matmuls.
                    Bt = main.tile([P, EC, T], f32r, tag="Xt")
                    Bn = main.tile([P, TC, EMB], bf16, tag="Bn")
                    FA = main.tile([P, HC, T], f32r, tag="FA")
                    bst = {}

                    def bdma(tp):
                        bst[tp] = pair_dma(B_d, b, tp)

                    def bcon(tp):
                        pair_consume(bst.pop(tp), Bt, tp, Bn)

                    if b == 0:
                        bdma(0)
                        bdma(1)
                    else:
                        bst[0], bst[1] = bn0, bn1
                    layer(w2f, b2t, H, FA, HC, weave=[
                        lambda: (bcon(0), bdma(2)), lambda: (bcon(1), bdma(3)),
                        lambda: bcon(2), lambda: bcon(3),
                        lambda: None, lambda: None,
                        lambda: None, lambda: None])

                    H2 = main.tile([P, HC, T], f32r, tag="H")
                    FB = main.tile([P, HC, T], f32r, tag="FB")
                    layer(w1f, b1t, Bt, H2, EC)
                    layer(w2f, b2t, H2, FB, HC)

                    # ---- e phase: one matmul pass, global stabilizer ----
                    Vx = main.tile([P, TC, T], bf16, tag="H")
                    Sca = main.tile([P, TC, T], bf16, tag="Xt")
                    rzb = main.tile([P, TC], f32, tag="rzb")
                    biast = main.tile([P, 1], f32, tag="biast")

                    def e_chunk(m, weave=None):
                        ps0 = psA.tile([P, 512], f32, tag="acc")
                        ps1 = psA.tile([P, 512], f32, tag="acc")
                        for k in range(HC):
                            st = FA[:, k, m * P:(m + 1) * P]
                            nc.tensor.matmul(ps0[:], st, FB[:, k, 0:512],
                                             start=(k == 0), stop=(k == HC - 1))
                            nc.tensor.matmul(ps1[:], st, FB[:, k, 512:1024],
                                             start=(k == 0), stop=(k == HC - 1))
                            if weave:
                                weave.pop(0)()
                        if m == 0:
                            # chunk-0 global max -> bias = OFFS - G0 on all
                            # partitions (gpsimd cross-partition reduce +
                            # broadcast; off the PE critical path)
                            r0 = stats.tile([P, 1], f32, tag="st")
                            r1 = stats.tile([P, 1], f32, tag="st")
                            nc.vector.tensor_reduce(r0[:], ps0[:], axis=AX.X,
                                                    op=OP.max)
                            nc.vector.tensor_reduce(r1[:], ps1[:], axis=AX.X,
                                                    op=OP.max)
                            rM = stats.tile([P, 1], f32r, tag="rm")
                            nc.vector.tensor_tensor(rM[:], r0[:], r1[:], OP.max)
                            # cross-partition max: PE transpose -> free-dim
                            # reduce -> (44 - G0) -> ones-row matmul broadcast
                            ptm = psT.tile([1, P], f32r, tag="tp")
                            nc.tensor.transpose(ptm[:], rM[:], idr[:])
                            g0 = stats.tile([1, 1], f32, tag="g0")
                            nc.vector.tensor_reduce(g0[:], ptm[:], axis=AX.X,
                                                    op=OP.max)
                            gb = stats.tile([1, 1], f32, tag="g0")
                            nc.vector.tensor_scalar(gb[:], g0[:], -1.0, OFFS,
                                                    OP.mult, OP.add)
                            gps = psT.tile([P, 1], f32, tag="tp")
                            nc.tensor.matmul(gps[:], ones1[:], gb[:],
                                             start=True, stop=True)
                            nc.vector.tensor_copy(biast[:], gps[:])
                        zz0 = stats.tile([P, 1], f32, tag="st")
                        zz1 = stats.tile([P, 1], f32, tag="st")
                        nc.scalar.activation(Vx[:, m, 0:512], ps0[:], AF.Exp,
                                             bias=biast[:], accum_out=zz0[:])
                        nc.scalar.activation(Vx[:, m, 512:1024], ps1[:], AF.Exp,
                                             bias=biast[:], accum_out=zz1[:])
                        zs = stats.tile([P, 1], f32, tag="st")
                        nc.vector.tensor_tensor(zs[:], zz0[:], zz1[:], OP.add)
                        nc.vector.reciprocal(rzb[:, m:m + 1], zs[:])

                    def t_ops(j):
                        # V[a-block j, c-block i] -> S_ca[c-chunk i, a-cols j]
                        ops = []
                        for i in range(TC):
                            def one(i=i):
                                pt = psT.tile([P, P], bf16, tag="tp", name="pt")
                                nc.tensor.transpose(
                                    pt[:], Vx[:, j, i * P:(i + 1) * P], idb[:])
                                nc.vector.tensor_copy(
                                    Sca[:, i, j * P:(j + 1) * P], pt[:])
                            ops.append(one)
                        return ops

                    ob_state = {}

                    def out_group(stT, mvT, oc, Out_d, w1, rz_ap, weave=None,
                                  alt=False, last=False):
                        pp = psT if alt else psA
                        tg = "tp" if alt else "acc"
                        ps0 = pp.tile([P, 384], f32, tag=tg, name="ps0")
                        ps1 = pp.tile([P, w1], f32, tag=tg, name="ps1")
                        for ck in range(TC):
                            st = stT[:, ck, oc * P:(oc + 1) * P]
                            nc.tensor.matmul(ps0[:], st, mvT[:, ck, 0:384],
                                             start=(ck == 0), stop=(ck == TC - 1))
                            nc.tensor.matmul(ps1[:], st, mvT[:, ck, 384:384 + w1],
                                             start=(ck == 0), stop=(ck == TC - 1))
                            if weave:
                                weave.pop(0)()
                        if rz_ap is None:
                            rz = stats.tile([P, 1], f32, tag="st")
                            nc.vector.reciprocal(rz[:], ps1[:, 384:385])
                        else:
                            rz = rz_ap
                        if oc % 2 == 0:
                            ob_state["ob"] = obp.tile([P, 2, EMB], f32, tag="ob",
                                                      name="ob")
                        ob = ob_state["ob"]
                        j2 = oc % 2
                        nc.scalar.activation(ob[:, j2, 0:384], ps0[:],
                                             AF.Copy, scale=rz[:])
                        nc.scalar.activation(ob[:, j2, 384:768], ps1[:, 0:384],
                                             AF.Copy, scale=rz[:])
                        if oc % 2 == 1:
                            if last:
                                nc.sync.dma_start(
                                    Out_d[b, (oc - 1) * P:oc * P, :]
                                    .rearrange("(c p) e -> p c e", p=P),
                                    ob[:, 0:1, :])
                                nc.scalar.dma_start(
                                    Out_d[b, oc * P:(oc + 1) * P, :]
                                    .rearrange("(c p) e -> p c e", p=P),
                                    ob[:, 1:2, :])
                            else:
                                nc.sync.dma_start(
                                    Out_d[b, (oc - 1) * P:(oc + 1) * P, :]
                                    .rearrange("(c p) e -> p c e", p=P), ob[:])

                    def beta_group(j, weave=None):
                        out_group(Sca, Bn, j, beta_d, 384, rzb[:, j:j + 1],
                                  weave=weave)

                    # e-region PE stream: transposes hide inside the e/beta
                    # chains; beta(j) follows T(j); exp latency never stalls
                    # the PE.
                    pad = [lambda: None] * 8
                    e_chunk(0)
                    e_chunk(1)
                    e_chunk(2, weave=t_ops(0))
                    beta_group(0, weave=t_ops(1))
                    e_chunk(3)
                    beta_group(1, weave=t_ops(2))
                    e_chunk(4)
                    beta_group(2, weave=t_ops(3))
                    e_chunk(5)
                    beta_group(3, weave=t_ops(4))
                    e_chunk(6)
                    beta_group(4, weave=t_ops(5))
                    e_chunk(7)
                    beta_group(5, weave=t_ops(6))
                    beta_group(6, weave=t_ops(7))
                    beta_group(7)

                    # alpha region; next-batch A pair DMAs + W2 reload launch
                    # under it (S_ca readers - the beta matmuls - are done)
                    a_dmas = []
                    if b + 1 < BL:
                        a_dmas.append(pair_dma(A_d, b + 1, 0))
                        a_dmas.append(pair_dma(A_d, b + 1, 1))
                    for oc in range(TC):
                        out_group(Vx, An, oc, alpha_d, 385, None,
                                  alt=(oc % 2 == 1),
                                  last=(b == BL - 1 and oc == 7))

                    # tail: next-batch A prep + L1(A); An/At/H slots are free
                    # once alpha is done
                    if b + 1 < BL:
                        At_n = main.tile([P, EC, T], f32r, tag="Xt")
                        An_n = main.tile([P, TC, EMB + 1], bf16, tag="An")
                        nc.gpsimd.memset(An_n[:, :, EMB:EMB + 1], 1.0)
                        pair_consume(a_dmas[0], At_n, 0, An_n)
                        d2 = pair_dma(A_d, b + 1, 2)
                        pair_consume(a_dmas[1], At_n, 1, An_n)
                        d3 = pair_dma(A_d, b + 1, 3)
                        pair_consume(d2, At_n, 2, An_n)
                        bdma_n0 = pair_dma(B_d, b + 1, 0)
                        pair_consume(d3, At_n, 3, An_n)
                        bdma_n1 = pair_dma(B_d, b + 1, 1)
                        H_n = main.tile([P, HC, T], f32r, tag="H")
                        layer(w1f, b1t, At_n, H_n, EC)
                        nextA = (At_n, An_n, H_n, bdma_n0, bdma_n1)

    _split_multi_waits(nc)
    return nc


def _get_nc():
    if "nc" not in _CACHE:
        _CACHE["nc"] = _build_nc()
    return _CACHE["nc"]


def kernel(A, B, W1, b1, W2, b2):
    from concourse.bass_utils import run_bass_kernel_spmd

    A = np.asarray(A, dtype=np.float32)
    B = np.asarray(B, dtype=np.float32)
    W1 = np.ascontiguousarray(np.asarray(W1, dtype=np.float32))
    b1 = np.ascontiguousarray(np.asarray(b1, dtype=np.float32))
    W2 = np.ascontiguousarray(np.asarray(W2, dtype=np.float32))
    b2 = np.ascontiguousarray(np.asarray(b2, dtype=np.float32))

    nc = _get_nc()
    in_maps = []
    for c in range(N_CORES):
        in_maps.append({
            "A": np.ascontiguousarray(A[c * BL:(c + 1) * BL]),
            "B": np.ascontiguousarray(B[c * BL:(c + 1) * BL]),
            "W1": W1, "b1": b1, "W2": W2, "b2": b2,
        })
    res = run_bass_kernel_spmd(nc, in_maps, core_ids=list(range(N_CORES)))
    beta = np.concatenate([res.results[c]["beta"] for c in range(N_CORES)], axis=0)
    alpha = np.concatenate([res.results[c]["alpha"] for c in range(N_CORES)], axis=0)
    return beta, alpha
